# revision 58
# baseline (speedup 1.0000x reference)
"""Trainium2 Bass kernel for multi-head self-attention.

Problem: B=4, S=2048, D=1024, H=16 heads (HD=64), fp32 I/O.
  qkv = x @ w_qkv + b_qkv ; attention(softmax(q k^T / 8) v) ; out @ w_out + b_out

Sharding over 8 NeuronCores: core c handles batch b=c//2 and heads
half=c%2 (8 heads each).  Each core computes a partial output
(its heads' contribution to out[b] @ w_out); the host sums the two
partials per batch and adds the constant bias terms.

Design notes (cost-model driven):
  - scores^T [keys, q] in fp16 (output-bound on the PE either way).
  - exp on ScalarE (the pacing engine: ~266 us of activation work).
  - PV computed TRANSPOSED: pvT[q, hd] = sum_k E[k, q] * v[k, hd]
    (lhsT = E tile, rhs = v).  Output free dim is only 65, so a full
    2048-key accumulation for 128 queries costs 16*65 rows instead of
    the 16*512 the untransposed form pays.  Row sums ride along as a
    fused ones-column in v; the softmax normalization becomes a
    per-partition scalar multiply on the DVE (no broadcast matmuls).
  - attn^T [q, f] is flipped back to attn [f, q] with DMA-engine
    transposes (XBAR), keeping the PE out of it.
  - QKV projections run as fp8e4 DoubleRow matmuls with an exact
    hi+lo split (x ~ x_hi + x_lo, w ~ w_hi + w_lo, dropping the
    lo*lo term): 3 half-rate passes = 75% of the fp16 cost, with
    ~0.1% relative error.  Scores/PV/out-proj stay fp16.
"""

import contextlib
import numpy as np
import ml_dtypes

import concourse.bacc as bacc
import concourse.tile as tile
from concourse.tile_rust import add_dep_helper
from concourse import mybir
from concourse.bass_utils import run_bass_kernel_spmd

B, S, D, H, HD = 4, 2048, 1024, 16, 64
NCORES = 8
NH = 8            # heads per core
QF = 512          # q features per core (= NH * HD), same for k and v
PC = 512          # position chunk (psum bank, fp32)
NPC = S // PC     # 4 position chunks
KT = S // 128     # 16 key-position tiles
QT = PC // 128    # 4 query tiles per position chunk
DC = D // 128     # 8 contraction chunks
DDC = DC // 2     # 4 double-contraction chunks (DoubleRow, K=256)
FT_QK = 8         # feature tiles of q+k (2*QF/128)
FT_AT = 4         # feature tiles of attn output (QF/128)

F32 = mybir.dt.float32
F16 = mybir.dt.float16
F8 = mybir.dt.float8e4
DR = mybir.MatmulPerfMode.DoubleRow

XS = 8.0          # host prescale for x (fp8 split)
WS = 128.0        # host prescale for w_qkv / w_v (fp8 split)
PSCALE = 1.0 / (XS * WS)   # undo on psum readout

_CACHE = {}


def _build(repeat=1):
    nc = bacc.Bacc("TRN2", target_bir_lowering=False, debug=False)

    # x / w_qk / w_v arrive host-permuted to the SBUF DoubleRow layout
    # (position-chunk-blocked for x) so every DMA lands as one contiguous
    # run per partition: [p, (pc,) ddc, t, *].
    xh = nc.dram_tensor("xh", [128, NPC, 2, DDC, 2, PC], F8,
                        kind="ExternalInput").ap()
    xk0h = nc.dram_tensor("xk0h", [128, 2, DDC, 2, 128], F8,
                          kind="ExternalInput").ap()
    wqkh = nc.dram_tensor("wqkh", [128, FT_QK, 2, DDC, 2, 128], F8,
                          kind="ExternalInput").ap()
    wvh = nc.dram_tensor("wvh", [128, 2, DDC, 2, QF], F8,
                         kind="ExternalInput").ap()
    bqk = nc.dram_tensor("bqk", [2 * QF, 1], F32, kind="ExternalInput").ap()
    wo = nc.dram_tensor("wo", [QF, D], F16, kind="ExternalInput").ap()
    out_d = nc.dram_tensor("out_partial", [S, D], F16, kind="ExternalOutput").ap()
    # the last q-chunk ships as two fp16 partials (head pairs 0-2 early,
    # head pair 3 in the epilogue) summed on the host, halving the DMA
    # bytes that sit behind the final exp
    o3a = nc.dram_tensor("out_q3a", [128, 4, D], F16, kind="ExternalOutput").ap()
    o3b = nc.dram_tensor("out_q3b", [128, 4, D], F16, kind="ExternalOutput").ap()

    with tile.TileContext(nc) as tc:
        with contextlib.ExitStack() as ctx:
            with nc.allow_low_precision(reason="fp16/fp8 intermediates are intentional"):
                token = None
                for _ in range(repeat):
                    token = _emit(nc, tc, ctx, xh, xk0h, wqkh, wvh,
                                  bqk, wo, out_d, o3a, o3b, token=token)
    nc.compile()
    return nc


def _emit(nc, tc, ctx, xh, xk0h, wqkh, wvh,
          bqk, wo, out_d, o3a, o3b, token=None):
    with contextlib.ExitStack() as kctx:
        return _emit_inner(nc, tc, kctx, xh, xk0h, wqkh, wvh,
                           bqk, wo, out_d, o3a, o3b, token)


def _emit_inner(nc, tc, ctx, xh, xk0h, wqkh, wvh,
                bqk, wo, out_d, o3a, o3b, token=None):
    # ---- long-lived SBUF tensors -------------------------------------------
    keep = ctx.enter_context(tc.tile_pool(name="keep", bufs=1))
    qkT = keep.tile([128, FT_QK, S], F16, tag="qkT")             # 32 KB/p
    v_sb = keep.tile([128, KT, NH, HD + 1], F16, tag="v_sb")     # 16.3 KB/p
    attn = keep.tile([128, FT_AT, S], F16, tag="attn")           # 16 KB/p
    E_sb0 = keep.tile([128, KT, 2, PC], F16, tag="E_sb0")        # 32 KB/p
    E_sb1 = keep.tile([128, KT, 2, PC], F16, tag="E_sb1")        # 32 KB/p
    xt = keep.tile([128, NPC, 2, DDC, 2, PC], F8, tag="xt")      # 32 KB/p
    xk0 = keep.tile([128, 2, DDC, 2, 128], F8, tag="xk0")        # 2 KB/p
    wv_t = keep.tile([128, 2, DDC, 2, QF], F8, tag="wv_t")       # 8 KB/p
    wo_t = keep.tile([128, FT_AT, D], F16, tag="wo_t")           # 8 KB/p
    bqk_t = keep.tile([128, FT_QK, 1], F32, tag="bqk")
    E_bufs = (E_sb0, E_sb1)

    wqk_pool = ctx.enter_context(tc.tile_pool(name="wqk_pool", bufs=2))
    at_pool = ctx.enter_context(tc.tile_pool(name="at_pool", bufs=2))
    rec_pool = ctx.enter_context(tc.tile_pool(name="rec_pool", bufs=2))
    stg = ctx.enter_context(tc.tile_pool(name="stg", bufs=2))
    ps = ctx.enter_context(tc.tile_pool(name="ps", bufs=1, space="PSUM"))

    def track(i):
        if token is not None:
            add_dep_helper(token.ins, i.ins, sync=True,
                           reason="serialize benchmark repeats")

    def emit_input_dmas():
        for pc in range(1, NPC):
            track(nc.sync.dma_start(out=xt[:, pc], in_=xh[:, pc]))
        track(nc.sync.dma_start(out=wv_t, in_=wvh))
        for fc in range(FT_AT):
            track(nc.sync.dma_start(
                out=wo_t[:, fc, :], in_=wo[fc * 128:(fc + 1) * 128, :]))

    def b1_dma(ft):
        """fetch one feature tile (hi+lo packed) of the qk weights."""
        wt = wqk_pool.tile([128, 2, DDC, 2, 128], F8, tag="wqk", name=f"wq{ft}")
        track(nc.sync.dma_start(out=wt, in_=wqkh[:, ft]))
        return wt

    def b1_mm(wt, ft, pc, lo=0, hi=PC):
        """qkT[f, s] = sum_d wqk[d, f] * xT[d, s] (+ bias), one pos chunk.

        3 DoubleRow passes (hi*hi, hi*lo, lo*hi), fp32 psum, one DVE
        op rescales by 2^-10 and adds the (prescaled) bias."""
        if (lo, hi) == (0, 128):
            xsl = lambda x_hl, dd: xk0[:, x_hl, dd, :, :]
        else:
            xsl = lambda x_hl, dd: xt[:, pc, x_hl, dd, :, lo:hi]
        qp = ps.tile([128, hi - lo], F32, tag="misc", bufs=2,
                     name=f"qp{ft}_{pc}_{lo}")
        passes = ((0, 0), (0, 1), (1, 0))
        n = 0
        for w_hl, x_hl in passes:
            for dd in range(DDC):
                nc.tensor.matmul(
                    qp, wt[:, w_hl, dd, :, :], xsl(x_hl, dd),
                    start=(n == 0), stop=(n == 3 * DDC - 1), perf_mode=DR)
                n += 1
        nc.vector.tensor_scalar(
            out=qkT[:, ft, pc * PC + lo:pc * PC + hi], in0=qp,
            scalar1=bqk_t[:, ft, :],
            scalar2=PSCALE, op0=mybir.AluOpType.add, op1=mybir.AluOpType.mult)

    def b2_st(st):
        """v[s, f] natural layout (+ ones column), one 128-position tile."""
        pc, so = st // 4, (st % 4) * 128
        vp = ps.tile([128, QF], F32, tag="misc", bufs=2, name=f"vp{st}")
        passes = ((0, 0), (0, 1), (1, 0))
        n = 0
        for x_hl, w_hl in passes:
            for dd in range(DDC):
                nc.tensor.matmul(
                    vp, xt[:, pc, x_hl, dd, :, so:so + 128],
                    wv_t[:, w_hl, dd, :, :],
                    start=(n == 0), stop=(n == 3 * DDC - 1), perf_mode=DR)
                n += 1
        nc.vector.tensor_scalar(
            out=v_sb[:, st, :, 0:HD],
            in0=vp.rearrange("p (h d) -> p h d", h=NH),
            scalar1=PSCALE, scalar2=None, op0=mybir.AluOpType.mult)

    def emit_sc(E_sb, pp, qc, kt):
        """scores^T matmuls + exp for one kt tile of block (pp, qc)."""
        kft = FT_AT + pp
        qft = pp
        qs = slice(qc * PC, (qc + 1) * PC)
        ks = slice(kt * 128, (kt + 1) * 128)
        sc = ps.tile([128, 2, PC], F32, tag="sc", bufs=2, name=f"sc{pp}_{qc}_{kt}")
        nc.tensor.matmul(
            sc[:, 0, :], qkT[0:64, kft, ks], qkT[0:64, qft, qs],
            start=True, stop=True)
        nc.tensor.matmul(
            sc[:, 1, :], qkT[64:128, kft, ks], qkT[64:128, qft, qs],
            start=True, stop=True)
        nc.scalar.activation(
            out=E_sb[:, kt, :, :], in_=sc,
            func=mybir.ActivationFunctionType.Exp, scale=0.125)

    def emit_pvT(E_sb, pvt, pp, kt):
        """transposed PV for one kt tile: pvT[q, hd] += E[k, q] v[k, hd].

        PSUM accumulation groups are zero-region (2 KB bank) granular, so
        the four (qt, h) slices sharing a bank form ONE group: start on the
        bank's first matmul (zeroing the whole region), stop on its last."""
        for qt in range(QT):
            for h in range(2):
                first = kt == 0 and qt % 2 == 0 and h == 0
                last = kt == KT - 1 and qt % 2 == 1 and h == 1
                nc.tensor.matmul(
                    pvt[:, qt, h, 0:HD + 1],
                    E_sb[:, kt, h, qt * 128:(qt + 1) * 128],
                    v_sb[:, kt, 2 * pp + h, :],
                    start=first, stop=last, skip_group_check=True)

    last_copy = None

    def d_st(qc, j):
        """output projection for one position tile of q-chunk qc."""
        nonlocal last_copy
        st = qc * 4 + j
        ss = slice(st * 128, (st + 1) * 128)
        ot = stg.tile([128, D], F16, tag="ot")
        for n in range(2):
            op = ps.tile([128, PC], F32, tag="misc", bufs=2, name=f"op{st}_{n}")
            for fc in range(FT_AT):
                nc.tensor.matmul(
                    op, attn[:, fc, ss], wo_t[:, fc, n * PC:(n + 1) * PC],
                    start=(fc == 0), stop=(fc == FT_AT - 1))
            last_copy = nc.vector.tensor_copy(
                out=ot[:, n * PC:(n + 1) * PC], in_=op)
            nc.sync.dma_start(out=out_d[ss, n * PC:(n + 1) * PC],
                              in_=ot[:, n * PC:(n + 1) * PC])

    pq3 = keep.tile([128, 4, D], F16, tag="pq3")   # qc3 partial (fc 0..2)

    def d3_partial(j, n):
        """head pairs 0..2 of the last q-chunk's projection, done early."""
        st = 3 * 4 + j
        ss = slice(st * 128, (st + 1) * 128)
        op = ps.tile([128, PC], F32, tag="misc", bufs=2, name=f"pp3_{j}_{n}")
        for fc in range(FT_AT - 1):
            nc.tensor.matmul(
                op, attn[:, fc, ss], wo_t[:, fc, n * PC:(n + 1) * PC],
                start=(fc == 0), stop=(fc == FT_AT - 2))
        nc.vector.tensor_copy(out=pq3[:, j, n * PC:(n + 1) * PC], in_=op)

    def d3_final(j):
        """last head pair of the last q-chunk, fused with the stored
        partial; ships fp16 (the host adds the two cores' partials)."""
        nonlocal last_copy
        st = 3 * 4 + j
        ss = slice(st * 128, (st + 1) * 128)
        ot = stg.tile([128, D], F16, tag="ot3")
        op = ps.tile([128, 2, PC], F32, tag="sc", bufs=2, name=f"fp3_{j}")
        for n in range(2):
            nc.tensor.matmul(
                op[:, n, :], attn[:, FT_AT - 1, ss],
                wo_t[:, FT_AT - 1, n * PC:(n + 1) * PC], start=True, stop=True)
        last_copy = nc.vector.tensor_add(
            out=ot, in0=op.rearrange("p a b -> p (a b)"), in1=pq3[:, j, :])
        nc.sync.dma_start(out=o3b[:, j, :], in_=ot)

    def emit_norm(pvt, pp, qc, on_act=False):
        """normalize with the fused row-sums and flip back to [f, s].  For
        the final block the scalar multiplies run split across DVE and the
        (by then idle) Act engine, with per-half transposes, so the epilogue
        chain starts as early as possible."""
        rec = rec_pool.tile([128, QT, 2, 1], F32, tag="rec")
        nc.vector.reciprocal(out=rec, in_=pvt[:, :, :, HD:HD + 1])
        at = at_pool.tile([128, QT, 2, HD], F16, tag="at")
        for qt in range(QT):
            for h in range(2):
                if on_act and qt >= 2:
                    nc.scalar.activation(
                        out=at[:, qt, h, :], in_=pvt[:, qt, h, 0:HD],
                        func=mybir.ActivationFunctionType.Copy,
                        scale=rec[:, qt, h, :])
                else:
                    nc.vector.tensor_scalar_mul(
                        out=at[:, qt, h, :], in0=pvt[:, qt, h, 0:HD],
                        scalar1=rec[:, qt, h, :])
        if on_act:
            for half in range(2):
                nc.sync.dma_start_transpose(
                    out=attn[:, pp,
                             qc * PC + half * 256:qc * PC + (half + 1) * 256
                             ].rearrange("p (qt q) -> p qt q", qt=2),
                    in_=at[:, 2 * half:2 * half + 2, :, :])
        else:
            nc.sync.dma_start_transpose(
                out=attn[:, pp, qc * PC:(qc + 1) * PC].rearrange(
                    "p (qt q) -> p qt q", qt=QT),
                in_=at)

    # ---- emission schedule --------------------------------------------------
    # Tile directs dependencies by emission order, so every producer must be
    # emitted before its consumer.  The Act engine (exp) is the pacing
    # resource: block i's PV^T interleaves with block i+1's scores+exp so
    # the exp stream never starves.  QKV projection units are spread as
    # fillers ahead of their first consumer.
    nc.vector.memset(v_sb[:, :, :, HD:HD + 1], 1.0)
    wts = {}
    wts[FT_AT] = b1_dma(FT_AT)     # k features of head pair 0
    wts[0] = b1_dma(0)             # q features of head pair 0
    track(nc.sync.dma_start(out=xk0, in_=xk0h))
    track(nc.sync.dma_start(
        out=bqk_t, in_=bqk.rearrange("(ft p) o -> p ft o", p=128)))
    track(nc.sync.dma_start(out=xt[:, 0], in_=xh[:, 0]))
    emit_input_dmas()
    b1_mm(wts[FT_AT], FT_AT, 0, 0, 128)   # k keys 0:128 (first sc tile)
    b1_mm(wts[0], 0, 0)                   # q positions 0:512
    for f in (FT_AT + 1, 1, FT_AT + 2, 2, FT_AT + 3, 3):
        wts[f] = b1_dma(f)

    # prologue: scores+exp of block 0; k chunks arrive just before needed,
    # q chunks for blocks 1-2 go early (they gate those blocks' scores),
    # v-projection units fill the remaining slack
    _prologue = {0: lambda: b1_mm(wts[FT_AT], FT_AT, 0, 128, PC),
                 1: lambda: b1_mm(wts[FT_AT], FT_AT, 1),
                 13: lambda: b1_mm(wts[0], 0, 1),
                 5: lambda: b1_mm(wts[FT_AT], FT_AT, 2),
                 9: lambda: b1_mm(wts[FT_AT], FT_AT, 3)}
    _b2_next = iter(range(6))
    for kt in range(KT):
        emit_sc(E_bufs[0], 0, 0, kt)
        fn = _prologue.get(kt)
        if fn is not None:
            fn()
        else:
            st = next(_b2_next, None)
            if st is not None:
                b2_st(st)          # v st 0..5

    # filler units keyed (block_index, kt_slot).  Emission-order deadlines:
    # sc for block j is emitted during block j-1, so q(pp, qc) must be
    # emitted before block 4pp+qc-1 starts, and k(pp, *) before block
    # 4pp-1 starts.  v st feeds pvT(0, kt=st) emitted at block 1 slot st+2.
    fillers = {}

    def add_filler(bi, kt_slot, fn):
        fillers.setdefault(bi, {}).setdefault(kt_slot, []).append(fn)

    # remaining v projection: st 6-10 in block 0, st 11-15 early in
    # block 1 (their pvT(0) consumers are deferred there too)
    for u, st in enumerate(range(6, 11)):
        add_filler(0, 2 * u + 1, lambda st=st: b2_st(st))
    add_filler(0, 13, lambda: b1_mm(wts[0], 0, 2))
    add_filler(1, 7, lambda: b1_mm(wts[0], 0, 3))
    for u, st in enumerate(range(11, 16)):
        add_filler(1, u, lambda st=st: b2_st(st))
    for pp in range(1, 4):
        base = 4 * pp
        add_filler(base - 2, 3, lambda pp=pp: b1_mm(wts[FT_AT + pp], FT_AT + pp, 0))
        add_filler(base - 2, 7, lambda pp=pp: b1_mm(wts[FT_AT + pp], FT_AT + pp, 1))
        add_filler(base - 2, 11, lambda pp=pp: b1_mm(wts[FT_AT + pp], FT_AT + pp, 2))
        add_filler(base - 2, 13, lambda pp=pp: b1_mm(wts[FT_AT + pp], FT_AT + pp, 3))
        add_filler(base - 2, 9, lambda pp=pp: b1_mm(wts[pp], pp, 0))
        add_filler(base - 1, 9, lambda pp=pp: b1_mm(wts[pp], pp, 1))
        add_filler(base + 0, 11, lambda pp=pp: b1_mm(wts[pp], pp, 2))
        add_filler(base + 1, 11, lambda pp=pp: b1_mm(wts[pp], pp, 3))
    # qc3 partial projection (fc 0..2) once attn[:, 0..2, qc3] is final
    for u in range(6):
        add_filler(12, 2 * u + 1, lambda u=u: d3_partial(u // 2, u % 2))
    add_filler(13, 1, lambda: d3_partial(3, 0))
    add_filler(13, 5, lambda: d3_partial(3, 1))
    # qc0..qc2 output projections spread over the last blocks
    add_filler(13, 9, lambda: d_st(0, 0))
    add_filler(13, 13, lambda: d_st(0, 1))
    add_filler(14, 3, lambda: d_st(0, 2))
    add_filler(14, 9, lambda: d_st(0, 3))
    add_filler(14, 13, lambda: d_st(1, 0))
    add_filler(15, 1, lambda: d_st(1, 1))
    add_filler(15, 3, lambda: d_st(1, 2))
    add_filler(15, 5, lambda: d_st(1, 3))
    add_filler(15, 7, lambda: d_st(2, 0))
    add_filler(15, 9, lambda: d_st(2, 1))
    add_filler(15, 11, lambda: d_st(2, 2))
    add_filler(15, 13, lambda: d_st(2, 3))

    # main loop: block i emits sc+exp for block i+1.  The PV^T stream runs
    # one block late through block 7 (so the front-loaded v/qk projections
    # don't starve the exp stream), catches up during block 8 (which has
    # filler slack), and runs in-block from block 9 on (so the final
    # normalizations + projections drain before the epilogue).  pv units
    # trail their exp by >= 2 slots so the in-order PE queue never parks
    # on an exp that has not run yet.
    blocks = [(pp, qc) for pp in range(NH // 2) for qc in range(NPC)]
    pvts = {}

    def pvt_for(j):
        if j not in pvts:
            pp_j, qc_j = blocks[j]
            pvts[j] = ps.tile([128, QT, 2, 128], F32, tag="pvT", bufs=1,
                              name=f"pv{pp_j}_{qc_j}")
        return pvts[j]

    def pv_unit(j, jkt):
        emit_pvT(E_bufs[j % 2], pvt_for(j), blocks[j][0], jkt)

    for i, (pp, qc) in enumerate(blocks):
        nxt = blocks[i + 1] if i + 1 < len(blocks) else None
        fl = fillers.get(i, {})
        for kt in range(KT):
            if nxt is not None:
                emit_sc(E_bufs[(i + 1) % 2], nxt[0], nxt[1], kt)
            for fn in fl.get(kt, []):
                fn()
            if i == 0:
                if 2 <= kt <= 11:
                    pv_unit(0, kt - 2)     # kts 0..9 only; rest deferred
            elif i == 1:
                if kt == 5:
                    for jkt in range(10, KT):
                        pv_unit(0, jkt)
                    emit_norm(pvt_for(0), 0, 0)
                elif kt >= 6:
                    pv_unit(1, kt - 6)
            elif kt >= 2:
                pv_unit(i, kt - 2)
        if i == 1:
            for jkt in range(KT - 6, KT):
                pv_unit(1, jkt)
            emit_norm(pvt_for(1), pp, qc)
        elif i >= 2:
            pv_unit(i, KT - 2)
            pv_unit(i, KT - 1)
            emit_norm(pvt_for(i), pp, qc, on_act=(i == len(blocks) - 1))
    # epilogue: the last q-chunk's projection (head pair 3 + stored partial)
    for j in range(4):
        d3_final(j)
    nc.sync.dma_start(out=o3a[:, 0, 0:1], in_=pq3[:, 0, 0:1])
    return last_copy


def _get_nc():
    if "nc" not in _CACHE:
        _CACHE["nc"] = _build()
    return _CACHE["nc"]


def _split8(a, scale):
    """exact-ish hi+lo fp8e4 split of `a*scale` (both at the same scale)."""
    s = (a * scale).astype(np.float32)
    hi = s.astype(ml_dtypes.float8_e4m3)
    lo = (s - hi.astype(np.float32)).astype(ml_dtypes.float8_e4m3)
    return hi, lo


def _dr_x(a):
    """[D, S] -> pc-blocked DoubleRow layout [128, NPC, DDC, 2, PC]."""
    # d = dd*256 + t*128 + p
    return np.ascontiguousarray(
        a.reshape(DDC, 2, 128, NPC, PC).transpose(2, 3, 0, 1, 4))


def _dr_w(a, tiled):
    """[D, F] -> DoubleRow layout [128, ft, DDC, 2, 128] (tiled) or
    [128, DDC, 2, F] (flat)."""
    f = a.shape[1]
    if tiled:
        return np.ascontiguousarray(
            a.reshape(DDC, 2, 128, f // 128, 128).transpose(2, 3, 0, 1, 4))
    return np.ascontiguousarray(
        a.reshape(DDC, 2, 128, f).transpose(2, 0, 1, 3))


def _make_in_maps(x, w_qkv, b_qkv, w_out):
    in_maps = []
    for c in range(NCORES):
        b, half = divmod(c, 2)
        hs = half * QF
        xT = np.ascontiguousarray(x[b].T)
        wqk = np.concatenate([w_qkv[:, hs:hs + QF],
                              w_qkv[:, D + hs:D + hs + QF]], axis=1)
        wv = np.ascontiguousarray(w_qkv[:, 2 * D + hs:2 * D + hs + QF])
        xhi, xlo = _split8(xT, XS)
        whi, wlo = _split8(wqk, WS)
        vhi, vlo = _split8(wv, WS)
        xhl = np.ascontiguousarray(
            np.stack([_dr_x(xhi), _dr_x(xlo)], axis=2))
        in_maps.append({
            "xh": xhl,
            "xk0h": np.ascontiguousarray(xhl[:, 0, :, :, :, 0:128]),
            "wqkh": np.ascontiguousarray(
                np.stack([_dr_w(whi, True), _dr_w(wlo, True)], axis=2)),
            "wvh": np.ascontiguousarray(
                np.stack([_dr_w(vhi, False), _dr_w(vlo, False)], axis=1)),
            "bqk": (np.concatenate([b_qkv[hs:hs + QF],
                                    b_qkv[D + hs:D + hs + QF]])[:, None]
                    / PSCALE).astype(np.float32),
            "wo": np.ascontiguousarray(w_out[hs:hs + QF, :]).astype(np.float16),
        })
    return in_maps


def kernel(x, w_qkv, b_qkv, w_out, b_out):
    x = np.asarray(x, dtype=np.float32)
    w_qkv = np.asarray(w_qkv, dtype=np.float32)
    b_qkv = np.asarray(b_qkv, dtype=np.float32)
    w_out = np.asarray(w_out, dtype=np.float32)
    b_out = np.asarray(b_out, dtype=np.float32)

    nc = _get_nc()
    in_maps = _make_in_maps(x, w_qkv, b_qkv, w_out)
    res = run_bass_kernel_spmd(nc, in_maps, list(range(NCORES)))
    _CACHE["last_results"] = res

    # host combine: out[b] = partial_A + partial_B + (b_out + bv @ w_out);
    # the last q-chunk arrives as two fp16 partials per core
    const = b_out + b_qkv[2 * D:] @ w_out            # [D]
    out = np.empty((B, S, D), dtype=np.float32)
    for b in range(B):
        ra, rb = res.results[2 * b], res.results[2 * b + 1]
        out[b, :3 * PC] = (ra["out_partial"][:3 * PC].astype(np.float32)
                           + rb["out_partial"][:3 * PC].astype(np.float32)
                           + const)
        q3 = (ra["out_q3b"].astype(np.float32)
              + rb["out_q3b"].astype(np.float32))
        out[b, 3 * PC:] = q3.transpose(1, 0, 2).reshape(PC, D) + const
    return out


# revision 59
# speedup vs baseline: 1.0084x; 1.0084x over previous
"""Trainium2 Bass kernel for multi-head self-attention.

Problem: B=4, S=2048, D=1024, H=16 heads (HD=64), fp32 I/O.
  qkv = x @ w_qkv + b_qkv ; attention(softmax(q k^T / 8) v) ; out @ w_out + b_out

Sharding over 8 NeuronCores: core c handles batch b=c//2 and heads
half=c%2 (8 heads each).  Each core computes a partial output
(its heads' contribution to out[b] @ w_out); the host sums the two
partials per batch and adds the constant bias terms.

Design notes (cost-model driven):
  - scores^T [keys, q] in fp16 (output-bound on the PE either way).
  - exp on ScalarE (the pacing engine: ~266 us of activation work).
  - PV computed TRANSPOSED: pvT[q, hd] = sum_k E[k, q] * v[k, hd]
    (lhsT = E tile, rhs = v).  Output free dim is only 65, so a full
    2048-key accumulation for 128 queries costs 16*65 rows instead of
    the 16*512 the untransposed form pays.  Row sums ride along as a
    fused ones-column in v; the softmax normalization becomes a
    per-partition scalar multiply on the DVE (no broadcast matmuls).
  - attn^T [q, f] is flipped back to attn [f, q] with DMA-engine
    transposes (XBAR), keeping the PE out of it.
  - QKV projections run as fp8e4 DoubleRow matmuls with an exact
    hi+lo split (x ~ x_hi + x_lo, w ~ w_hi + w_lo, dropping the
    lo*lo term): 3 half-rate passes = 75% of the fp16 cost, with
    ~0.1% relative error.  Scores/PV/out-proj stay fp16.
"""

import contextlib
import numpy as np
import ml_dtypes

import concourse.bacc as bacc
import concourse.tile as tile
from concourse.tile_rust import add_dep_helper
from concourse import mybir
from concourse.bass_utils import run_bass_kernel_spmd

B, S, D, H, HD = 4, 2048, 1024, 16, 64
NCORES = 8
NH = 8            # heads per core
QF = 512          # q features per core (= NH * HD), same for k and v
PC = 512          # position chunk (psum bank, fp32)
NPC = S // PC     # 4 position chunks
KT = S // 128     # 16 key-position tiles
QT = PC // 128    # 4 query tiles per position chunk
DC = D // 128     # 8 contraction chunks
DDC = DC // 2     # 4 double-contraction chunks (DoubleRow, K=256)
FT_QK = 8         # feature tiles of q+k (2*QF/128)
FT_AT = 4         # feature tiles of attn output (QF/128)

F32 = mybir.dt.float32
F16 = mybir.dt.float16
F8 = mybir.dt.float8e4
DR = mybir.MatmulPerfMode.DoubleRow

XS = 8.0          # host prescale for x (fp8 split)
WS = 128.0        # host prescale for w_qkv / w_v (fp8 split)
PSCALE = 1.0 / (XS * WS)   # undo on psum readout

_CACHE = {}


def _build(repeat=1):
    nc = bacc.Bacc("TRN2", target_bir_lowering=False, debug=False)

    # x / w_qk / w_v arrive host-permuted to the SBUF DoubleRow layout
    # (position-chunk-blocked for x) so every DMA lands as one contiguous
    # run per partition: [p, (pc,) ddc, t, *].
    xh = nc.dram_tensor("xh", [128, NPC, 2, DDC, 2, PC], F8,
                        kind="ExternalInput").ap()
    xk0h = nc.dram_tensor("xk0h", [128, 2, DDC, 2, 128], F8,
                          kind="ExternalInput").ap()
    wqkh = nc.dram_tensor("wqkh", [128, FT_QK, 2, DDC, 2, 128], F8,
                          kind="ExternalInput").ap()
    wvh = nc.dram_tensor("wvh", [128, 2, DDC, 2, QF], F8,
                         kind="ExternalInput").ap()
    bqk = nc.dram_tensor("bqk", [2 * QF, 1], F32, kind="ExternalInput").ap()
    idm = nc.dram_tensor("idm", [128, 128], F16, kind="ExternalInput").ap()
    wo = nc.dram_tensor("wo", [QF, D], F16, kind="ExternalInput").ap()
    out_d = nc.dram_tensor("out_partial", [S, D], F16, kind="ExternalOutput").ap()
    # the last q-chunk ships as two fp16 partials (head pairs 0-2 early,
    # head pair 3 in the epilogue) summed on the host, halving the DMA
    # bytes that sit behind the final exp
    o3a = nc.dram_tensor("out_q3a", [128, 4, D], F16, kind="ExternalOutput").ap()
    o3b = nc.dram_tensor("out_q3b", [128, 4, D], F16, kind="ExternalOutput").ap()

    with tile.TileContext(nc) as tc:
        with contextlib.ExitStack() as ctx:
            with nc.allow_low_precision(reason="fp16/fp8 intermediates are intentional"):
                token = None
                for _ in range(repeat):
                    token = _emit(nc, tc, ctx, xh, xk0h, wqkh, wvh,
                                  bqk, idm, wo, out_d, o3a, o3b, token=token)
    nc.compile()
    return nc


def _emit(nc, tc, ctx, xh, xk0h, wqkh, wvh,
          bqk, idm, wo, out_d, o3a, o3b, token=None):
    with contextlib.ExitStack() as kctx:
        return _emit_inner(nc, tc, kctx, xh, xk0h, wqkh, wvh,
                           bqk, idm, wo, out_d, o3a, o3b, token)


def _emit_inner(nc, tc, ctx, xh, xk0h, wqkh, wvh,
                bqk, idm, wo, out_d, o3a, o3b, token=None):
    # ---- long-lived SBUF tensors -------------------------------------------
    keep = ctx.enter_context(tc.tile_pool(name="keep", bufs=1))
    qkT = keep.tile([128, FT_QK, S], F16, tag="qkT")             # 32 KB/p
    v_sb = keep.tile([128, KT, NH, HD + 1], F16, tag="v_sb")     # 16.3 KB/p
    attn = keep.tile([128, FT_AT, S], F16, tag="attn")           # 16 KB/p
    E_sb0 = keep.tile([128, KT, 2, PC], F16, tag="E_sb0")        # 32 KB/p
    E_sb1 = keep.tile([128, KT, 2, PC], F16, tag="E_sb1")        # 32 KB/p
    xt = keep.tile([128, NPC, 2, DDC, 2, PC], F8, tag="xt")      # 32 KB/p
    xk0 = keep.tile([128, 2, DDC, 2, 128], F8, tag="xk0")        # 2 KB/p
    wv_t = keep.tile([128, 2, DDC, 2, QF], F8, tag="wv_t")       # 8 KB/p
    wo_t = keep.tile([128, FT_AT, D], F16, tag="wo_t")           # 8 KB/p
    bqk_t = keep.tile([128, FT_QK, 1], F32, tag="bqk")
    id_t = keep.tile([128, 128], F16, tag="ident")
    E_bufs = (E_sb0, E_sb1)

    wqk_pool = ctx.enter_context(tc.tile_pool(name="wqk_pool", bufs=2))
    at_pool = ctx.enter_context(tc.tile_pool(name="at_pool", bufs=2))
    rec_pool = ctx.enter_context(tc.tile_pool(name="rec_pool", bufs=2))
    stg = ctx.enter_context(tc.tile_pool(name="stg", bufs=2))
    ps = ctx.enter_context(tc.tile_pool(name="ps", bufs=1, space="PSUM"))

    def track(i):
        if token is not None:
            add_dep_helper(token.ins, i.ins, sync=True,
                           reason="serialize benchmark repeats")

    def emit_input_dmas():
        for pc in range(1, NPC):
            track(nc.sync.dma_start(out=xt[:, pc], in_=xh[:, pc]))
        track(nc.sync.dma_start(out=wv_t, in_=wvh))
        for fc in range(FT_AT):
            track(nc.sync.dma_start(
                out=wo_t[:, fc, :], in_=wo[fc * 128:(fc + 1) * 128, :]))
        track(nc.sync.dma_start(out=id_t, in_=idm))

    def b1_dma(ft):
        """fetch one feature tile (hi+lo packed) of the qk weights."""
        wt = wqk_pool.tile([128, 2, DDC, 2, 128], F8, tag="wqk", name=f"wq{ft}")
        track(nc.sync.dma_start(out=wt, in_=wqkh[:, ft]))
        return wt

    def b1_mm(wt, ft, pc, lo=0, hi=PC):
        """qkT[f, s] = sum_d wqk[d, f] * xT[d, s] (+ bias), one pos chunk.

        3 DoubleRow passes (hi*hi, hi*lo, lo*hi), fp32 psum, one DVE
        op rescales by 2^-10 and adds the (prescaled) bias."""
        if (lo, hi) == (0, 128):
            xsl = lambda x_hl, dd: xk0[:, x_hl, dd, :, :]
        else:
            xsl = lambda x_hl, dd: xt[:, pc, x_hl, dd, :, lo:hi]
        qp = ps.tile([128, hi - lo], F32, tag="misc", bufs=2,
                     name=f"qp{ft}_{pc}_{lo}")
        passes = ((0, 0), (0, 1), (1, 0))
        n = 0
        for w_hl, x_hl in passes:
            for dd in range(DDC):
                nc.tensor.matmul(
                    qp, wt[:, w_hl, dd, :, :], xsl(x_hl, dd),
                    start=(n == 0), stop=(n == 3 * DDC - 1), perf_mode=DR)
                n += 1
        nc.vector.tensor_scalar(
            out=qkT[:, ft, pc * PC + lo:pc * PC + hi], in0=qp,
            scalar1=bqk_t[:, ft, :],
            scalar2=PSCALE, op0=mybir.AluOpType.add, op1=mybir.AluOpType.mult)

    def b2_st(st):
        """v[s, f] natural layout (+ ones column), one 128-position tile."""
        pc, so = st // 4, (st % 4) * 128
        vp = ps.tile([128, QF], F32, tag="misc", bufs=2, name=f"vp{st}")
        passes = ((0, 0), (0, 1), (1, 0))
        n = 0
        for x_hl, w_hl in passes:
            for dd in range(DDC):
                nc.tensor.matmul(
                    vp, xt[:, pc, x_hl, dd, :, so:so + 128],
                    wv_t[:, w_hl, dd, :, :],
                    start=(n == 0), stop=(n == 3 * DDC - 1), perf_mode=DR)
                n += 1
        nc.vector.tensor_scalar(
            out=v_sb[:, st, :, 0:HD],
            in0=vp.rearrange("p (h d) -> p h d", h=NH),
            scalar1=PSCALE, scalar2=None, op0=mybir.AluOpType.mult)

    def emit_sc(E_sb, pp, qc, kt):
        """scores^T matmuls + exp for one kt tile of block (pp, qc)."""
        kft = FT_AT + pp
        qft = pp
        qs = slice(qc * PC, (qc + 1) * PC)
        ks = slice(kt * 128, (kt + 1) * 128)
        sc = ps.tile([128, 2, PC], F32, tag="sc", bufs=2, name=f"sc{pp}_{qc}_{kt}")
        nc.tensor.matmul(
            sc[:, 0, :], qkT[0:64, kft, ks], qkT[0:64, qft, qs],
            start=True, stop=True)
        nc.tensor.matmul(
            sc[:, 1, :], qkT[64:128, kft, ks], qkT[64:128, qft, qs],
            start=True, stop=True)
        nc.scalar.activation(
            out=E_sb[:, kt, :, :], in_=sc,
            func=mybir.ActivationFunctionType.Exp, scale=0.125)

    def emit_pvT(E_sb, pvt, pp, kt):
        """transposed PV for one kt tile: pvT[q, hd] += E[k, q] v[k, hd].

        PSUM accumulation groups are zero-region (2 KB bank) granular, so
        the four (qt, h) slices sharing a bank form ONE group: start on the
        bank's first matmul (zeroing the whole region), stop on its last."""
        for qt in range(QT):
            for h in range(2):
                first = kt == 0 and qt % 2 == 0 and h == 0
                last = kt == KT - 1 and qt % 2 == 1 and h == 1
                nc.tensor.matmul(
                    pvt[:, qt, h, 0:HD + 1],
                    E_sb[:, kt, h, qt * 128:(qt + 1) * 128],
                    v_sb[:, kt, 2 * pp + h, :],
                    start=first, stop=last, skip_group_check=True)

    last_copy = None

    def d_st(qc, j):
        """output projection for one position tile of q-chunk qc."""
        nonlocal last_copy
        st = qc * 4 + j
        ss = slice(st * 128, (st + 1) * 128)
        ot = stg.tile([128, D], F16, tag="ot")
        for n in range(2):
            op = ps.tile([128, PC], F32, tag="misc", bufs=2, name=f"op{st}_{n}")
            for fc in range(FT_AT):
                nc.tensor.matmul(
                    op, attn[:, fc, ss], wo_t[:, fc, n * PC:(n + 1) * PC],
                    start=(fc == 0), stop=(fc == FT_AT - 1))
            last_copy = nc.vector.tensor_copy(
                out=ot[:, n * PC:(n + 1) * PC], in_=op)
            nc.sync.dma_start(out=out_d[ss, n * PC:(n + 1) * PC],
                              in_=ot[:, n * PC:(n + 1) * PC])

    pq3 = keep.tile([128, 4, D], F16, tag="pq3")   # qc3 partial (fc 0..2)

    def d3_partial(j, n):
        """head pairs 0..2 of the last q-chunk's projection, done early."""
        st = 3 * 4 + j
        ss = slice(st * 128, (st + 1) * 128)
        op = ps.tile([128, PC], F32, tag="misc", bufs=2, name=f"pp3_{j}_{n}")
        for fc in range(FT_AT - 1):
            nc.tensor.matmul(
                op, attn[:, fc, ss], wo_t[:, fc, n * PC:(n + 1) * PC],
                start=(fc == 0), stop=(fc == FT_AT - 2))
        nc.vector.tensor_copy(out=pq3[:, j, n * PC:(n + 1) * PC], in_=op)

    def d3_final(j):
        """last head pair of the last q-chunk, fused with the stored
        partial; ships fp16 (the host adds the two cores' partials)."""
        nonlocal last_copy
        st = 3 * 4 + j
        ss = slice(st * 128, (st + 1) * 128)
        ot = stg.tile([128, D], F16, tag="ot3")
        op = ps.tile([128, 2, PC], F32, tag="sc", bufs=2, name=f"fp3_{j}")
        for n in range(2):
            nc.tensor.matmul(
                op[:, n, :], attn[:, FT_AT - 1, ss],
                wo_t[:, FT_AT - 1, n * PC:(n + 1) * PC], start=True, stop=True)
        last_copy = nc.vector.tensor_add(
            out=ot, in0=op.rearrange("p a b -> p (a b)"), in1=pq3[:, j, :])
        nc.sync.dma_start(out=o3b[:, j, :], in_=ot)

    def emit_norm(pvt, pp, qc, on_act=False):
        """normalize with the fused row-sums and flip back to [f, s].  For
        the final block the scalar multiplies run split across DVE and the
        (by then idle) Act engine, with per-half transposes, so the epilogue
        chain starts as early as possible."""
        rec = rec_pool.tile([128, QT, 2, 1], F32, tag="rec")
        nc.vector.reciprocal(out=rec, in_=pvt[:, :, :, HD:HD + 1])
        at = at_pool.tile([128, QT, 2, HD], F16, tag="at")
        for qt in range(QT):
            for h in range(2):
                if on_act and qt >= 2:
                    nc.scalar.activation(
                        out=at[:, qt, h, :], in_=pvt[:, qt, h, 0:HD],
                        func=mybir.ActivationFunctionType.Copy,
                        scale=rec[:, qt, h, :])
                else:
                    nc.vector.tensor_scalar_mul(
                        out=at[:, qt, h, :], in0=pvt[:, qt, h, 0:HD],
                        scalar1=rec[:, qt, h, :])
            if on_act:
                # PE transpose + DVE copy: ~0.3us per q-tile instead of the
                # ~2.3us DMA-transpose dispatch chain, and each projection
                # tile unblocks as soon as its own q-tile is normalized
                tp = ps.tile([128, 128], F16, tag="misc", bufs=2,
                             name=f"tp{qt}")
                nc.tensor.transpose(
                    tp, at[:, qt, :, :].rearrange("p a b -> p (a b)"), id_t)
                nc.vector.tensor_copy(
                    out=attn[:, pp, qc * PC + qt * 128:qc * PC + (qt + 1) * 128],
                    in_=tp)
        if not on_act:
            nc.sync.dma_start_transpose(
                out=attn[:, pp, qc * PC:(qc + 1) * PC].rearrange(
                    "p (qt q) -> p qt q", qt=QT),
                in_=at)

    # ---- emission schedule --------------------------------------------------
    # Tile directs dependencies by emission order, so every producer must be
    # emitted before its consumer.  The Act engine (exp) is the pacing
    # resource: block i's PV^T interleaves with block i+1's scores+exp so
    # the exp stream never starves.  QKV projection units are spread as
    # fillers ahead of their first consumer.
    nc.vector.memset(v_sb[:, :, :, HD:HD + 1], 1.0)
    wts = {}
    wts[FT_AT] = b1_dma(FT_AT)     # k features of head pair 0
    wts[0] = b1_dma(0)             # q features of head pair 0
    track(nc.sync.dma_start(out=xk0, in_=xk0h))
    track(nc.sync.dma_start(
        out=bqk_t, in_=bqk.rearrange("(ft p) o -> p ft o", p=128)))
    track(nc.sync.dma_start(out=xt[:, 0], in_=xh[:, 0]))
    emit_input_dmas()
    b1_mm(wts[FT_AT], FT_AT, 0, 0, 128)   # k keys 0:128 (first sc tile)
    b1_mm(wts[0], 0, 0)                   # q positions 0:512
    for f in (FT_AT + 1, 1, FT_AT + 2, 2, FT_AT + 3, 3):
        wts[f] = b1_dma(f)

    # prologue: scores+exp of block 0; k chunks arrive just before needed,
    # q chunks for blocks 1-2 go early (they gate those blocks' scores),
    # v-projection units fill the remaining slack
    _prologue = {0: lambda: b1_mm(wts[FT_AT], FT_AT, 0, 128, PC),
                 1: lambda: b1_mm(wts[FT_AT], FT_AT, 1),
                 13: lambda: b1_mm(wts[0], 0, 1),
                 5: lambda: b1_mm(wts[FT_AT], FT_AT, 2),
                 9: lambda: b1_mm(wts[FT_AT], FT_AT, 3)}
    _b2_next = iter(range(6))
    for kt in range(KT):
        emit_sc(E_bufs[0], 0, 0, kt)
        fn = _prologue.get(kt)
        if fn is not None:
            fn()
        else:
            st = next(_b2_next, None)
            if st is not None:
                b2_st(st)          # v st 0..5

    # filler units keyed (block_index, kt_slot).  Emission-order deadlines:
    # sc for block j is emitted during block j-1, so q(pp, qc) must be
    # emitted before block 4pp+qc-1 starts, and k(pp, *) before block
    # 4pp-1 starts.  v st feeds pvT(0, kt=st) emitted at block 1 slot st+2.
    fillers = {}

    def add_filler(bi, kt_slot, fn):
        fillers.setdefault(bi, {}).setdefault(kt_slot, []).append(fn)

    # remaining v projection: st 6-10 in block 0, st 11-15 early in
    # block 1 (their pvT(0) consumers are deferred there too)
    for u, st in enumerate(range(6, 11)):
        add_filler(0, 2 * u + 1, lambda st=st: b2_st(st))
    add_filler(0, 13, lambda: b1_mm(wts[0], 0, 2))
    add_filler(1, 7, lambda: b1_mm(wts[0], 0, 3))
    for u, st in enumerate(range(11, 16)):
        add_filler(1, u, lambda st=st: b2_st(st))
    for pp in range(1, 4):
        base = 4 * pp
        add_filler(base - 2, 3, lambda pp=pp: b1_mm(wts[FT_AT + pp], FT_AT + pp, 0))
        add_filler(base - 2, 7, lambda pp=pp: b1_mm(wts[FT_AT + pp], FT_AT + pp, 1))
        add_filler(base - 2, 11, lambda pp=pp: b1_mm(wts[FT_AT + pp], FT_AT + pp, 2))
        add_filler(base - 2, 13, lambda pp=pp: b1_mm(wts[FT_AT + pp], FT_AT + pp, 3))
        add_filler(base - 2, 9, lambda pp=pp: b1_mm(wts[pp], pp, 0))
        add_filler(base - 1, 9, lambda pp=pp: b1_mm(wts[pp], pp, 1))
        add_filler(base + 0, 11, lambda pp=pp: b1_mm(wts[pp], pp, 2))
        add_filler(base + 1, 11, lambda pp=pp: b1_mm(wts[pp], pp, 3))
    # qc3 partial projection (fc 0..2) once attn[:, 0..2, qc3] is final
    for u in range(6):
        add_filler(12, 2 * u + 1, lambda u=u: d3_partial(u // 2, u % 2))
    add_filler(13, 1, lambda: d3_partial(3, 0))
    add_filler(13, 5, lambda: d3_partial(3, 1))
    # qc0..qc2 output projections spread over the last blocks
    add_filler(13, 9, lambda: d_st(0, 0))
    add_filler(13, 13, lambda: d_st(0, 1))
    add_filler(14, 3, lambda: d_st(0, 2))
    add_filler(14, 9, lambda: d_st(0, 3))
    add_filler(14, 13, lambda: d_st(1, 0))
    add_filler(15, 1, lambda: d_st(1, 1))
    add_filler(15, 3, lambda: d_st(1, 2))
    add_filler(15, 5, lambda: d_st(1, 3))
    add_filler(15, 7, lambda: d_st(2, 0))
    add_filler(15, 9, lambda: d_st(2, 1))
    add_filler(15, 11, lambda: d_st(2, 2))
    add_filler(15, 13, lambda: d_st(2, 3))

    # main loop: block i emits sc+exp for block i+1.  The PV^T stream runs
    # one block late through block 7 (so the front-loaded v/qk projections
    # don't starve the exp stream), catches up during block 8 (which has
    # filler slack), and runs in-block from block 9 on (so the final
    # normalizations + projections drain before the epilogue).  pv units
    # trail their exp by >= 2 slots so the in-order PE queue never parks
    # on an exp that has not run yet.
    blocks = [(pp, qc) for pp in range(NH // 2) for qc in range(NPC)]
    pvts = {}

    def pvt_for(j):
        if j not in pvts:
            pp_j, qc_j = blocks[j]
            pvts[j] = ps.tile([128, QT, 2, 128], F32, tag="pvT", bufs=1,
                              name=f"pv{pp_j}_{qc_j}")
        return pvts[j]

    def pv_unit(j, jkt):
        emit_pvT(E_bufs[j % 2], pvt_for(j), blocks[j][0], jkt)

    for i, (pp, qc) in enumerate(blocks):
        nxt = blocks[i + 1] if i + 1 < len(blocks) else None
        fl = fillers.get(i, {})
        for kt in range(KT):
            if nxt is not None:
                emit_sc(E_bufs[(i + 1) % 2], nxt[0], nxt[1], kt)
            for fn in fl.get(kt, []):
                fn()
            if i == 0:
                if 2 <= kt <= 11:
                    pv_unit(0, kt - 2)     # kts 0..9 only; rest deferred
            elif i == 1:
                if kt == 5:
                    for jkt in range(10, KT):
                        pv_unit(0, jkt)
                    emit_norm(pvt_for(0), 0, 0)
                elif kt >= 6:
                    pv_unit(1, kt - 6)
            elif kt >= 2:
                pv_unit(i, kt - 2)
        if i == 1:
            for jkt in range(KT - 6, KT):
                pv_unit(1, jkt)
            emit_norm(pvt_for(1), pp, qc)
        elif i >= 2:
            pv_unit(i, KT - 2)
            pv_unit(i, KT - 1)
            emit_norm(pvt_for(i), pp, qc, on_act=(i == len(blocks) - 1))
    # epilogue: the last q-chunk's projection (head pair 3 + stored partial)
    for j in range(4):
        d3_final(j)
    nc.sync.dma_start(out=o3a[:, 0, 0:1], in_=pq3[:, 0, 0:1])
    return last_copy


def _get_nc():
    if "nc" not in _CACHE:
        _CACHE["nc"] = _build()
    return _CACHE["nc"]


def _split8(a, scale):
    """exact-ish hi+lo fp8e4 split of `a*scale` (both at the same scale)."""
    s = (a * scale).astype(np.float32)
    hi = s.astype(ml_dtypes.float8_e4m3)
    lo = (s - hi.astype(np.float32)).astype(ml_dtypes.float8_e4m3)
    return hi, lo


def _dr_x(a):
    """[D, S] -> pc-blocked DoubleRow layout [128, NPC, DDC, 2, PC]."""
    # d = dd*256 + t*128 + p
    return np.ascontiguousarray(
        a.reshape(DDC, 2, 128, NPC, PC).transpose(2, 3, 0, 1, 4))


def _dr_w(a, tiled):
    """[D, F] -> DoubleRow layout [128, ft, DDC, 2, 128] (tiled) or
    [128, DDC, 2, F] (flat)."""
    f = a.shape[1]
    if tiled:
        return np.ascontiguousarray(
            a.reshape(DDC, 2, 128, f // 128, 128).transpose(2, 3, 0, 1, 4))
    return np.ascontiguousarray(
        a.reshape(DDC, 2, 128, f).transpose(2, 0, 1, 3))


def _make_in_maps(x, w_qkv, b_qkv, w_out):
    in_maps = []
    for c in range(NCORES):
        b, half = divmod(c, 2)
        hs = half * QF
        xT = np.ascontiguousarray(x[b].T)
        wqk = np.concatenate([w_qkv[:, hs:hs + QF],
                              w_qkv[:, D + hs:D + hs + QF]], axis=1)
        wv = np.ascontiguousarray(w_qkv[:, 2 * D + hs:2 * D + hs + QF])
        xhi, xlo = _split8(xT, XS)
        whi, wlo = _split8(wqk, WS)
        vhi, vlo = _split8(wv, WS)
        xhl = np.ascontiguousarray(
            np.stack([_dr_x(xhi), _dr_x(xlo)], axis=2))
        in_maps.append({
            "xh": xhl,
            "xk0h": np.ascontiguousarray(xhl[:, 0, :, :, :, 0:128]),
            "wqkh": np.ascontiguousarray(
                np.stack([_dr_w(whi, True), _dr_w(wlo, True)], axis=2)),
            "wvh": np.ascontiguousarray(
                np.stack([_dr_w(vhi, False), _dr_w(vlo, False)], axis=1)),
            "idm": np.eye(128, dtype=np.float16),
            "bqk": (np.concatenate([b_qkv[hs:hs + QF],
                                    b_qkv[D + hs:D + hs + QF]])[:, None]
                    / PSCALE).astype(np.float32),
            "wo": np.ascontiguousarray(w_out[hs:hs + QF, :]).astype(np.float16),
        })
    return in_maps


def kernel(x, w_qkv, b_qkv, w_out, b_out):
    x = np.asarray(x, dtype=np.float32)
    w_qkv = np.asarray(w_qkv, dtype=np.float32)
    b_qkv = np.asarray(b_qkv, dtype=np.float32)
    w_out = np.asarray(w_out, dtype=np.float32)
    b_out = np.asarray(b_out, dtype=np.float32)

    nc = _get_nc()
    in_maps = _make_in_maps(x, w_qkv, b_qkv, w_out)
    res = run_bass_kernel_spmd(nc, in_maps, list(range(NCORES)))
    _CACHE["last_results"] = res

    # host combine: out[b] = partial_A + partial_B + (b_out + bv @ w_out);
    # the last q-chunk arrives as two fp16 partials per core
    const = b_out + b_qkv[2 * D:] @ w_out            # [D]
    out = np.empty((B, S, D), dtype=np.float32)
    for b in range(B):
        ra, rb = res.results[2 * b], res.results[2 * b + 1]
        out[b, :3 * PC] = (ra["out_partial"][:3 * PC].astype(np.float32)
                           + rb["out_partial"][:3 * PC].astype(np.float32)
                           + const)
        q3 = (ra["out_q3b"].astype(np.float32)
              + rb["out_q3b"].astype(np.float32))
        out[b, 3 * PC:] = q3.transpose(1, 0, 2).reshape(PC, D) + const
    return out
